# revision 23
# baseline (speedup 1.0000x reference)
"""Trainium2 Bass kernel for DeformationTrackerBiFlowModel (non-teacher-forcing).

Reference math (per batch element b, per step t):
    x_t   = [prev_out (2), fin_t (3)]            (5,)
    h_t   = tanh(x_t @ W_rnn + b_rnn)            (12,)   (U_rnn is inert: h0 == 0)
    out_t = [cp0 (2), h_t (12)] @ W_out + b_out  (2,)
    prev_out_{t+1} = out_t;  prev_out_0 = cp0

The autoregressive chain limits wall time to T * (per-step dependency
latency), so the recurrence is refactored to keep only TWO ops on the chain
(one matmul -> one tanh):

    psum_t = [fin_t, 1, cp0] @ wF                 [mmF, start=True, off chain]
           + h_{t-1} @ wHO                        [mmHO, accumulate, ON chain]
    rows 0:96   = pre_t  -> h_t = tanh(...)       [ACT, ON chain]
    rows 96:112 = h_{t-1} @ Wo2 -> out_{t-1} = that + (cp0 @ Wo1 + b_out)
                                                  [DVE add, off chain]

  where W1p = W_rnn[:2], Wo1 = W_out[:2], Wo2 = W_out[2:]; wF packs W_rnn[2:],
  a ones-row carrying (b_rnn + b_out @ W1p), cp0-rows carrying (Wo1 @ W1p),
  and zero columns for the out region; wHO packs [Wo2 @ W1p | Wo2]. Step 0
  uses wF0 (direct W1p / b_rnn rows); a final step T runs mmHO alone to emit
  out_{T-1}. Dummy matmuls keep the PE's HAM activity window busy so it runs
  at 2.4 GHz instead of the cold 1.2 GHz.

Device mapping: batch 65536 sharded over 8 cores (8192 each = G*C*COLS
exactly); features on SBUF partitions with G=8 trajectories packed
block-diagonally per matmul; C=2 independent column chains of COLS=512.
"""

import os
from contextlib import ExitStack

import numpy as np

import concourse.mybir as mybir
import concourse.tile as tile
from concourse import bacc
from concourse.bass_utils import run_bass_kernel_spmd

B, T = 65536, 100
D_CP, D_FIN, HID = 2, 3, 12
NCORES = 8
BC = B // NCORES              # 8192 per core
G = 8                         # trajectories packed per matmul (block-diag)
C = 2                         # independent column chains
COLS = 512                    # batch columns per chain; G*C*COLS == BC
XROWS = D_FIN * G + 1 + D_CP * G   # fin rows + ones row + cp0 rows = 41
MPRE = HID * G                # 96: pre/h rows
MOUT = D_CP * G               # 16: out rows (psum rows 96:112)
MTOT = MPRE + MOUT            # 112

F32 = mybir.dt.float32

# Matmul-path dtype: bf16 streams 1 col/cycle on the PE; float32r ~1.6-2
# cycles/col; exact fp32 4 cycles/col. DTB_MM in {bf16, f32r, f32}.
_MM_CHOICES = {"bf16": mybir.dt.bfloat16, "f32r": mybir.dt.float32r, "f32": F32}
MM_DTYPE = _MM_CHOICES[os.environ.get("DTB_MM", "bf16")]
MM_NP = mybir.dt.np(MM_DTYPE)

N_DUMMIES = int(os.environ.get("DTB_DUMMIES", "0"))  # keepalive MMs per chain-step

LAST_RESULTS = None  # test.py introspects profiling info from here


def build_program(t_steps=T, g=G, c=C, cols=COLS, mm_dtype=None, n_dummies=None):
    if mm_dtype is None:
        mm_dtype = MM_DTYPE
    if n_dummies is None:
        n_dummies = N_DUMMIES
    XDT = mm_dtype
    xrows = D_FIN * g + 1 + D_CP * g
    mpre, mout = HID * g, D_CP * g
    mtot = mpre + mout
    nc = bacc.Bacc(target_bir_lowering=False)

    fin = nc.dram_tensor("fin", [t_steps, c, D_FIN * g, cols], XDT, kind="ExternalInput")
    xc = nc.dram_tensor("xc", [c, xrows - D_FIN * g, cols], XDT, kind="ExternalInput")
    cb = nc.dram_tensor("cb", [c, mout, cols], F32, kind="ExternalInput")
    wf = nc.dram_tensor("wf", [xrows, mtot], XDT, kind="ExternalInput")
    wf0 = nc.dram_tensor("wf0", [xrows, mtot], XDT, kind="ExternalInput")
    who = nc.dram_tensor("who", [mpre, mtot], XDT, kind="ExternalInput")
    out = nc.dram_tensor("out", [t_steps, c, mout, cols], XDT, kind="ExternalOutput")

    tanh = mybir.ActivationFunctionType.Tanh

    with tile.TileContext(nc) as tc, ExitStack() as ctx:
        const = ctx.enter_context(tc.tile_pool(name="const", bufs=1))
        xpool = ctx.enter_context(tc.tile_pool(name="xpool", bufs=1))
        hpool = ctx.enter_context(tc.tile_pool(name="hpool", bufs=3))
        opool = ctx.enter_context(tc.tile_pool(name="opool", bufs=3))
        # bufs=2: cycling PSUM groups over more banks re-throttles the PE's
        # HAM clock gate (observed: 379ns warm matmuls decay to 672ns cold).
        psum = ctx.enter_context(tc.tile_pool(name="psum", bufs=2, space="PSUM"))
        if n_dummies:
            dpsum = ctx.enter_context(tc.tile_pool(name="dpsum", bufs=2, space="PSUM"))

        wfs = const.tile([xrows, mtot], XDT, name="wfs")
        nc.sync.dma_start(out=wfs, in_=wf[:, :])
        wf0s = const.tile([xrows, mtot], XDT, name="wf0s")
        nc.sync.dma_start(out=wf0s, in_=wf0[:, :])
        whos = const.tile([mpre, mtot], XDT, name="whos")
        nc.sync.dma_start(out=whos, in_=who[:, :])
        cbs = []
        for ch in range(c):
            cbt = const.tile([mout, cols], F32, tag=f"cb{ch}", name=f"cbs{ch}")
            nc.sync.dma_start(out=cbt, in_=cb[ch])
            cbs.append(cbt)

        # Keepalive dummy matmul operands (always ready, junk output).
        if n_dummies:
            dw = const.tile([1, 1], XDT, name="dw")
            nc.vector.memset(dw, 0)
            dx = const.tile([1, 64], XDT, name="dx")
            nc.vector.memset(dx, 0)

        # Two persistent x tiles per chain (even/odd step): the ones+cp0 rows
        # are written once; the per-step DMA ships only the fin rows, and the
        # WAR dep on mmF(t-2) gives the DMA a two-step lead.
        nfin = D_FIN * g
        xtiles = {}
        for ch in range(c):
            for par in range(2):
                xt = xpool.tile([xrows, cols], XDT, tag=f"x{ch}_{par}", name=f"x_{ch}_{par}")
                nc.sync.dma_start(out=xt[nfin:, :], in_=xc[ch])
                if par < t_steps:
                    nc.sync.dma_start(out=xt[0:nfin, :], in_=fin[par, ch])
                xtiles[(ch, par)] = xt
        xts = {(ch, tt): xtiles[(ch, tt % 2)] for ch in range(c) for tt in range(t_steps)}

        # Group for step 0 is mmF0 alone (no h_{-1}).
        p1s = []
        for ch in range(c):
            p1 = psum.tile([mtot, cols], F32, tag=f"p1{ch}", name=f"p1_{ch}_0")
            nc.tensor.matmul(p1, wf0s, xts[(ch, 0)], start=True, stop=True)
            p1s.append(p1)

        hs = [None] * c
        for t in range(t_steps + 1):
            # Open group t+1 FIRST: the PE is in-order, so ready mmF work must
            # sit ahead of the chain-blocked mmHO closers in its queue.
            p1n = list(p1s)
            if 0 < t + 1 < t_steps:
                for ch in range(c):
                    p1 = psum.tile([mtot, cols], F32, tag=f"p1{ch}", name=f"p1_{ch}_{t + 1}")
                    nc.tensor.matmul(p1, wfs, xts[(ch, t + 1)], start=True, stop=False)
                    p1n[ch] = p1

            # Close group t: mmHO is the only cross-step dependency.
            if 0 < t < t_steps:
                for ch in range(c):
                    nc.tensor.matmul(p1s[ch], whos, hs[ch], start=False, stop=True)
            elif t == t_steps:
                for ch in range(c):
                    p1 = psum.tile([mtot, cols], F32, tag=f"p1{ch}", name=f"p1_{ch}_{t}")
                    nc.tensor.matmul(p1, whos, hs[ch], start=True, stop=True)
                    p1s[ch] = p1

            for ch in range(c):
                if t < t_steps:
                    h = hpool.tile([mpre, cols], XDT, tag=f"h{ch}", name=f"h_{ch}_{t}")
                    nc.scalar.activation(h, p1s[ch][0:mpre, :], tanh)
                    hs[ch] = h
                if t > 0:
                    osb = opool.tile([mout, cols], XDT, tag=f"o{ch}", name=f"o_{ch}_{t}")
                    nc.vector.tensor_add(osb, p1s[ch][mpre:mtot, :], cbs[ch])
                    nc.gpsimd.dma_start(out=out[t - 1, ch], in_=osb)

                # PE keepalive (optional): tiny always-ready matmuls.
                for di in range(n_dummies):
                    dp = dpsum.tile([1, 64], F32, tag="dum", name=f"d_{t}_{ch}_{di}")
                    nc.tensor.matmul(dp, dw, dx, start=True, stop=True)

                # Prefetch fin for iteration t+2's opener (same-parity tile).
                if t + 2 < t_steps:
                    nc.sync.dma_start(
                        out=xtiles[(ch, t % 2)][0:nfin, :], in_=fin[t + 2, ch]
                    )

            p1s = p1n
    nc.compile()
    return nc


def build_packed_weights(W_rnn, W_out, b_rnn, b_out, g=G):
    W_rnn = np.asarray(W_rnn, np.float32)
    W_out = np.asarray(W_out, np.float32)
    b_rnn = np.asarray(b_rnn, np.float32)
    b_out = np.asarray(b_out, np.float32)
    W1p, W1f = W_rnn[:D_CP], W_rnn[D_CP:]
    Wo1, Wo2 = W_out[:D_CP], W_out[D_CP:]
    xrows = D_FIN * g + 1 + D_CP * g
    mpre, mout = HID * g, D_CP * g
    mtot = mpre + mout
    ones_row = D_FIN * g

    wf = np.zeros((xrows, mtot), np.float32)
    wf0 = np.zeros((xrows, mtot), np.float32)
    who = np.zeros((mpre, mtot), np.float32)
    E = Wo1 @ W1p                      # (2, 12) cp0 contribution to next pre
    r = b_rnn + b_out @ W1p            # (12,) ones-row weight (steady state)
    Wh = Wo2 @ W1p                     # (12, 12) h contribution to next pre
    for i in range(g):
        hsl = slice(HID * i, HID * (i + 1))
        wf[D_FIN * i : D_FIN * (i + 1), hsl] = W1f
        wf0[D_FIN * i : D_FIN * (i + 1), hsl] = W1f
        wf[ones_row, hsl] = r
        wf0[ones_row, hsl] = b_rnn
        csl = slice(ones_row + 1 + D_CP * i, ones_row + 1 + D_CP * (i + 1))
        wf[csl, hsl] = E
        wf0[csl, hsl] = W1p
        who[hsl, hsl] = Wh
        who[hsl, mpre + D_CP * i : mpre + D_CP * (i + 1)] = Wo2
    return wf, wf0, who


def stage_inputs(cp0, fin, cvec, g=G, c=C, cols=COLS, t_steps=T):
    """Per-core staging: batch-major -> feature-major device layouts.

    fin_d rows per (t, chain): [fin (3G) | ones (1) | cp0 (2G)].
    """
    bp = g * c * cols
    bc = cp0.shape[0]
    fin_p = np.zeros((bp, t_steps, D_FIN), np.float32)
    fin_p[:bc] = fin
    cp0_p = np.zeros((bp, D_CP), np.float32)
    cp0_p[:bc] = cp0
    cv_p = np.zeros((bp, D_CP), np.float32)
    cv_p[:bc] = cvec
    # b = ch*(g*cols) + gi*cols + j
    fin_d = np.ascontiguousarray(
        fin_p.reshape(c, g, cols, t_steps, D_FIN).transpose(3, 0, 1, 4, 2)
    ).reshape(t_steps, c, D_FIN * g, cols)
    # constant x rows: [ones (1) | cp0 (2G)]
    xc_d = np.ones((c, 1 + D_CP * g, cols), np.float32)
    xc_d[:, 1:, :] = cp0_p.reshape(c, g, cols, D_CP).transpose(0, 1, 3, 2).reshape(
        c, D_CP * g, cols
    )
    cb_d = np.ascontiguousarray(
        cv_p.reshape(c, g, cols, D_CP).transpose(0, 1, 3, 2)
    ).reshape(c, D_CP * g, cols)
    return fin_d, xc_d, cb_d


def unstage_output(out_d, bc, g=G, c=C, cols=COLS, t_steps=T):
    """(T, C, 2G, COLS) device layout -> (bc, T, 2) batch-major."""
    bp = g * c * cols
    o = out_d.reshape(t_steps, c, g, D_CP, cols).transpose(1, 2, 4, 0, 3)
    return np.ascontiguousarray(o).reshape(bp, t_steps, D_CP)[:bc]


def kernel(control_point_input, finger_input, W_rnn, U_rnn, b_rnn, W_out, b_out):
    global LAST_RESULTS
    cp = np.asarray(control_point_input, np.float32)
    fin = np.asarray(finger_input, np.float32)
    W_rnn = np.asarray(W_rnn, np.float32)
    b_rnn = np.asarray(b_rnn, np.float32)
    W_out = np.asarray(W_out, np.float32)
    b_out = np.asarray(b_out, np.float32)

    cp0 = cp[:, 0, :]                                  # (B, 2)
    cvec = cp0 @ W_out[:D_CP] + b_out                  # (B, 2), constant per step
    wf, wf0, who = build_packed_weights(W_rnn, W_out, b_rnn, b_out)
    wf, wf0, who = (x.astype(MM_NP) for x in (wf, wf0, who))

    nc = build_program()
    in_maps = []
    for m in range(NCORES):
        sl = slice(m * BC, (m + 1) * BC)
        fin_d, xc_d, cb_d = stage_inputs(cp0[sl], fin[sl], cvec[sl])
        in_maps.append(
            {"fin": fin_d.astype(MM_NP, copy=False),
             "xc": xc_d.astype(MM_NP, copy=False), "cb": cb_d,
             "wf": wf, "wf0": wf0, "who": who}
        )

    trace = bool(os.environ.get("DTB_TRACE"))
    res = run_bass_kernel_spmd(
        nc, in_maps, core_ids=list(range(NCORES)), trace=trace
    )
    LAST_RESULTS = res

    outs = [
        unstage_output(np.asarray(res.results[m]["out"], np.float32), BC)
        for m in range(NCORES)
    ]
    return np.concatenate(outs, axis=0)
